# revision 17
# baseline (speedup 1.0000x reference)
"""CoverageAttention fused Trainium2 kernel (8 NeuronCores, data-parallel over batch).

Computation (per batch b):
  energy[s,h] = tanh( enc[b] @ W_h + dec_proj[b] + coverage[b,s]*W_c )
  scores[s]   = energy[s,:] @ v  (+ mask bias)
  attn        = softmax(scores); coverage_new = coverage + attn
  context     = attn @ enc[b]

v3 strategy. The per-call cost of executing this kernel through the PJRT path
is dominated by a fixed per-IO-TENSOR dispatch overhead (~11 ms/tensor,
measured), not by bytes or instruction count. So the kernel takes exactly ONE
input tensor (a packed bf16 blob holding the pre-transposed encoder, weights,
biases, coverage and mask rows) and produces ONE output tensor
(out[3, bpc, S] = ctx / attn / coverage_new rows).

Device-side (per core, 8 batches/core):
  - Encoder shipped once, in bf16, host-pre-tiled to per-(batch, s-block)
    contiguous [128, KT*SB] tiles so every DMA is a single 16 KiB/partition
    contiguous transfer.
  - Main matmul computes energy TRANSPOSED: psum[h:128, s:512] =
    sum_k wh[k].T @ encT[k] in bf16 (1 cycle/row on PE), plus one K=1 matmul
    adding coverage[s]*W_c[h] as an outer product.
  - tanh + dec_proj bias fused in one ScalarE activation per tile (dec_proj is
    computed on the host and shipped as a per-partition bias table).
  - scores accumulate on PE: psum[1,512] += v.T @ energy over 8 h-tiles, then
    one K=1 matmul adds the precomputed mask bias row ((mask-1)*1e4).
  - softmax on the [1, 2048] row: reduce_max(negate) -> Exp activation with
    bias=-max and fused accum_out denominator -> reciprocal -> scale.
  - context is computed WITHOUT re-reading the encoder: the batch's encT
    s-block tiles stay resident in SBUF (6-buffer pool); attn is broadcast to
    all 128 partitions with a K=1 PE matmul (ones ⊗ attn) and 64 DVE
    multiply + free-dim-reduce pairs produce ctxT[e], finished by a small PE
    transpose to lay the context out as a row.
  - Batches software-pipelined: tail(b) is emitted after main(b+1)'s first
    s-block so softmax+context hide under the next batch's matmuls while
    keeping the encoder-tile pool race-free.
"""

import numpy as np

P = 128
B_FULL = 64
S_FULL = 2048
H = 1024
E = 2048
N_CORES = 8
SB = 512           # s-block (matmul free dim / psum bank)
KT = E // P        # 16 contraction tiles over encoder dim
HT = H // P        # 8 h-tiles
NSB = S_FULL // SB # 4 s-blocks per batch
BPC = B_FULL // N_CORES

_CACHE: dict = {}


def _blob_offsets(bpc: int, S: int):
    """Element offsets of each logical block inside the packed bf16 blob."""
    nsb = S // SB
    off = {}
    cur = 0

    def add(name, n):
        nonlocal cur
        off[name] = cur
        cur += n

    add("enct", bpc * nsb * P * KT * SB)
    add("wh", P * KT * H)
    add("wc", H)
    add("vv", P * HT)
    add("dp", P * HT * bpc)
    add("covr", bpc * S)
    add("maskb", bpc * S)
    add("ones", P)
    add("ident", P * P)
    return off, cur


def _build_nc(bpc: int, S: int):
    import concourse.mybir as mybir
    import concourse.tile as tile
    from concourse import bacc

    f32 = mybir.dt.float32
    bf16 = mybir.dt.bfloat16
    Tanh = mybir.ActivationFunctionType.Tanh
    Exp = mybir.ActivationFunctionType.Exp
    Copy = mybir.ActivationFunctionType.Copy
    X = mybir.AxisListType.X

    nsb = S // SB
    off, total = _blob_offsets(bpc, S)

    # enable_partition_id=False: the kernel never reads the core id (all
    # per-core data is prepared host-side), and every NEFF io tensor costs
    # ~11 ms of per-call dispatch overhead on this setup.
    nc = bacc.Bacc(None, target_bir_lowering=False, enable_partition_id=False)

    blob = nc.dram_tensor("blob", [total], bf16, kind="ExternalInput")
    out_o = nc.dram_tensor("out_o", [3, bpc, S], f32, kind="ExternalOutput")

    def bslice(name, n, shape_expr, **axes):
        ap = blob[off[name]:off[name] + n]
        return ap.rearrange(shape_expr, **axes)

    with tile.TileContext(nc) as tc:
        with (
            tc.tile_pool(name="weights", bufs=1) as weights,
            tc.tile_pool(name="etp", bufs=6) as etp,
            tc.tile_pool(name="epool", bufs=2) as epool,
            tc.tile_pool(name="rows", bufs=1) as rows,
            tc.tile_pool(name="rows2", bufs=2) as rows2,
            tc.tile_pool(name="singles", bufs=1) as singles,
            tc.tile_pool(name="mainps", bufs=2, space="PSUM") as mainps,
            tc.tile_pool(name="scoresps", bufs=2, space="PSUM") as scoresps,
            tc.tile_pool(name="bcps", bufs=2, space="PSUM") as bcps,
            tc.tile_pool(name="ctxps", bufs=2, space="PSUM") as ctxps,
        ):
            # ---------------- prologue: weights ----------------
            # wh is DMA'd in 4 k-chunks so the first matmul group only waits
            # for ~1 MB instead of the whole 4 MB table.
            wh_sb = weights.tile([P, KT, H], bf16, tag="wh")
            wh_src = bslice("wh", P * KT * H, "(p k h) -> p k h", p=P, h=H)
            for c in range(4):
                kc = KT // 4
                nc.sync.dma_start(
                    wh_sb[:, c * kc:(c + 1) * kc, :],
                    wh_src[:, c * kc:(c + 1) * kc, :])
            wc_sb = singles.tile([1, H], bf16)
            nc.sync.dma_start(wc_sb, bslice("wc", H, "(o h) -> o h", o=1))
            v_sb = singles.tile([P, HT], bf16)
            nc.sync.dma_start(v_sb, bslice("vv", P * HT, "(p t) -> p t", p=P))
            one_sb = singles.tile([1, P], bf16)
            nc.sync.dma_start(one_sb, bslice("ones", P, "(o p) -> o p", o=1))
            dp_sb = singles.tile([P, HT, bpc], bf16)
            nc.sync.dma_start(
                dp_sb,
                bslice("dp", P * HT * bpc, "(p h b) -> p h b", p=P, b=bpc))
            id_sb = singles.tile([P, P], bf16)
            nc.sync.dma_start(
                id_sb, bslice("ident", P * P, "(p q) -> p q", p=P))
            # f32 identity (exact convert) so the ctx transpose is a pure-f32
            # matmul — the BIR verifier rejects mixed f32/bf16 matmul inputs.
            idf_sb = singles.tile([P, P], f32)
            nc.scalar.copy(idf_sb, id_sb)

            state = {}

            def emit_main_sblock(b, sb, ets):
                covr_row, maskb_row, scores_row = state[b][1:4]
                et = etp.tile([P, KT, SB], bf16, tag="et")
                nc.sync.dma_start(
                    et,
                    blob[off["enct"] + (b * nsb + sb) * P * KT * SB:
                         off["enct"] + (b * nsb + sb + 1) * P * KT * SB]
                    .rearrange("(p k s) -> p k s", p=P, s=SB),
                )
                ets.append(et)
                sc_ps = scoresps.tile([1, SB], f32, tag="scoresps")
                for ht in range(HT):
                    mp = mainps.tile([P, SB], f32, tag="mainps")
                    for k in range(KT):
                        nc.tensor.matmul(
                            mp,
                            wh_sb[:, k, ht * P:(ht + 1) * P],
                            et[:, k, :],
                            start=(k == 0),
                            stop=False,
                        )
                    # += coverage[s] * W_c[h]  (outer product, K=1)
                    nc.tensor.matmul(
                        mp,
                        wc_sb[0:1, ht * P:(ht + 1) * P],
                        covr_row[0:1, sb * SB:(sb + 1) * SB],
                        start=False,
                        stop=True,
                    )
                    en = epool.tile([P, SB], bf16, tag="energy")
                    nc.scalar.activation(
                        en, mp, Tanh, bias=dp_sb[:, ht, b:b + 1]
                    )
                    nc.tensor.matmul(
                        sc_ps,
                        v_sb[:, ht:ht + 1],
                        en,
                        start=(ht == 0),
                        stop=False,
                    )
                # += (mask-1)*1e4
                nc.tensor.matmul(
                    sc_ps,
                    one_sb[0:1, 0:1],
                    maskb_row[0:1, sb * SB:(sb + 1) * SB],
                    start=False,
                    stop=True,
                )
                nc.scalar.copy(scores_row[0:1, sb * SB:(sb + 1) * SB], sc_ps)

            def begin_main(b):
                # bufs=2: batch b+1's row DMAs must not wait on batch b's last
                # readers (a ~6 us PE stall per batch otherwise).
                covr_row = rows.tile([1, S], bf16, tag="covr", bufs=2)
                nc.sync.dma_start(
                    covr_row,
                    blob[off["covr"] + b * S: off["covr"] + (b + 1) * S]
                    .rearrange("(o s) -> o s", o=1))
                maskb_row = rows.tile([1, S], bf16, tag="maskb", bufs=2)
                nc.sync.dma_start(
                    maskb_row,
                    blob[off["maskb"] + b * S: off["maskb"] + (b + 1) * S]
                    .rearrange("(o s) -> o s", o=1))
                scores_row = rows2.tile([1, S], f32, tag="scores")
                ets = []
                state[b] = (covr_row, covr_row, maskb_row, scores_row, ets)

            def emit_tail(b):
                covr_row, _, _, scores_row, ets = state.pop(b)
                nmax = singles.tile([1, 1], f32, tag=f"nmax{b % 2}")
                nc.vector.reduce_max(nmax, scores_row[0:1, :], axis=X,
                                     negate=True)
                attn_u = rows.tile([1, S], f32, tag="attnu")
                den = singles.tile([1, 1], f32, tag=f"den{b % 2}")
                nc.scalar.activation(
                    attn_u, scores_row[0:1, :], Exp, bias=nmax[0:1, 0:1],
                    accum_out=den[0:1, 0:1],
                )
                rden = singles.tile([1, 1], f32, tag=f"rden{b % 2}")
                nc.vector.reciprocal(rden, den)
                attn_rb = rows.tile([1, S], bf16, tag="attnrb")
                nc.vector.tensor_scalar_mul(attn_rb, attn_u, rden[0:1, 0:1])
                # attn (f32): in-place scale of attn_u
                nc.vector.tensor_scalar_mul(attn_u, attn_u, rden[0:1, 0:1])
                nc.sync.dma_start(
                    out_o[1, b].rearrange("(o s) -> o s", o=1), attn_u)
                # coverage_new = coverage + attn
                covn_row = rows.tile([1, S], f32, tag="covn")
                nc.vector.tensor_add(covn_row, covr_row, attn_u)
                nc.sync.dma_start(
                    out_o[2, b].rearrange("(o s) -> o s", o=1), covn_row)
                # context from the SBUF-resident encT tiles: broadcast attn to
                # all 128 partitions via a K=1 PE matmul (ones ⊗ attn_row),
                # then per (kt, s-block): DVE multiply + free-dim reduce.
                pf = rows2.tile([P, KT, nsb], f32, tag="pf")
                for sb in range(nsb):
                    et = ets[sb]
                    bc = bcps.tile([P, SB], f32, tag="bcps")
                    nc.tensor.matmul(
                        bc,
                        one_sb[0:1, :],
                        attn_rb[0:1, sb * SB:(sb + 1) * SB],
                        start=True,
                        stop=True,
                    )
                    for kt in range(KT):
                        # multiply on DVE, free-dim reduce on ScalarE
                        # (activation Copy + accum_out) so the two engines
                        # pipeline and the last batch's drain is halved.
                        prod = rows.tile([P, SB], bf16, tag="ttscratch",
                                         bufs=2)
                        nc.vector.tensor_mul(prod, et[:, kt, :], bc)
                        dump = rows.tile([P, SB], bf16, tag="actdump")
                        nc.scalar.activation(
                            dump, prod, Copy,
                            accum_out=pf[:, kt, sb:sb + 1])
                ctxc = rows2.tile([P, KT], f32, tag="ctxc")
                nc.vector.reduce_sum(ctxc, pf[:, :, :], axis=X)
                ctp = ctxps.tile([KT, P], f32, tag="ctxps")
                nc.tensor.transpose(ctp, ctxc, idf_sb)
                ctx_sb = rows2.tile([KT, P], f32, tag="ctxsb")
                nc.scalar.copy(ctx_sb, ctp)
                nc.sync.dma_start(
                    out_o[0, b].rearrange("(k p) -> k p", k=KT), ctx_sb)

            # software pipeline: tail(b-1) lands after main(b)'s first s-block
            # so the et pool (6 bufs) never reuses a slot before its tail
            # readers are emitted.
            begin_main(0)
            for sb in range(nsb):
                emit_main_sblock(0, sb, state[0][4])
            for b in range(1, bpc):
                begin_main(b)
                emit_main_sblock(b, 0, state[b][4])
                emit_tail(b - 1)
                for sb in range(1, nsb):
                    emit_main_sblock(b, sb, state[b][4])
            emit_tail(bpc - 1)

    nc.compile()
    return nc


def _get_nc(bpc: int, S: int):
    key = (bpc, S)
    if key not in _CACHE:
        _CACHE[key] = _build_nc(bpc, S)
    return _CACHE[key]


def _prepare_in_maps(decoder_hidden, encoder_outputs, coverage, mask,
                     W_h, W_d, W_c, v, n_cores: int):
    """Host-side prep: shard over batch, pack one bf16 blob per core."""
    import ml_dtypes
    bf16 = ml_dtypes.bfloat16

    dec = np.asarray(decoder_hidden, dtype=np.float32)
    cov = np.asarray(coverage, dtype=np.float32)
    msk = np.asarray(mask)
    B = dec.shape[0]
    S = cov.shape[1]
    nsb = S // SB
    bpc = B // n_cores
    off, total = _blob_offsets(bpc, S)

    wh_np = np.asarray(W_h, dtype=np.float32)          # [E, H]
    wh_b = np.ascontiguousarray(
        wh_np.reshape(KT, P, H).transpose(1, 0, 2)).astype(bf16).ravel()
    wc_b = np.asarray(W_c, dtype=np.float32).astype(bf16).ravel()
    v_np = np.asarray(v, dtype=np.float32)[:, 0]
    v_b = np.ascontiguousarray(v_np.reshape(HT, P).T).astype(bf16).ravel()
    dec_proj = (dec.astype(np.float64) @ np.asarray(W_d, np.float64))
    dec_proj = dec_proj.astype(np.float32)             # [B, H]
    maskb = ((msk.astype(np.float32) - 1.0) * 10000.0).astype(bf16)
    ones_b = np.ones(P, dtype=np.float32).astype(bf16)
    ident_b = np.eye(P, dtype=np.float32).astype(bf16).ravel()

    enc = np.asarray(encoder_outputs, dtype=np.float32)
    in_maps = []
    for c in range(n_cores):
        sl = slice(c * bpc, (c + 1) * bpc)
        # [bpc, S, E] -> encT [bpc, E, S] -> [bpc, KT, P, nsb, SB]
        # -> [bpc, nsb, P, KT, SB]
        enct = (
            enc[sl]
            .transpose(0, 2, 1)
            .reshape(bpc, KT, P, nsb, SB)
            .transpose(0, 3, 2, 1, 4)
        )
        dslice = dec_proj[sl]                              # [bpc, H]
        dp_c = np.ascontiguousarray(
            dslice.T.reshape(HT, P, bpc).transpose(1, 0, 2))

        blob = np.empty(total, dtype=bf16)
        blob[off["enct"]:off["enct"] + enct.size] = (
            np.ascontiguousarray(enct).astype(bf16).ravel())
        blob[off["wh"]:off["wh"] + wh_b.size] = wh_b
        blob[off["wc"]:off["wc"] + wc_b.size] = wc_b
        blob[off["vv"]:off["vv"] + v_b.size] = v_b
        blob[off["dp"]:off["dp"] + dp_c.size] = (
            dp_c.astype(bf16).ravel())
        blob[off["covr"]:off["covr"] + bpc * S] = (
            cov[sl].astype(bf16).ravel())
        blob[off["maskb"]:off["maskb"] + bpc * S] = maskb[sl].ravel()
        blob[off["ones"]:off["ones"] + P] = ones_b
        blob[off["ident"]:off["ident"] + P * P] = ident_b
        in_maps.append({"blob": blob})
    return in_maps, bpc


def kernel(decoder_hidden, encoder_outputs, coverage, mask, W_h, W_d, W_c, v):
    from concourse.bass_utils import run_bass_kernel_spmd

    in_maps, bpc = _prepare_in_maps(
        decoder_hidden, encoder_outputs, coverage, mask, W_h, W_d, W_c, v,
        N_CORES,
    )
    S = np.asarray(coverage).shape[1]
    nc = _get_nc(bpc, S)
    res = run_bass_kernel_spmd(nc, in_maps, core_ids=list(range(N_CORES)))
    context = np.concatenate([r["out_o"][0] for r in res.results], axis=0)
    attn = np.concatenate([r["out_o"][1] for r in res.results], axis=0)
    covn = np.concatenate([r["out_o"][2] for r in res.results], axis=0)
    return context, attn, covn


# revision 19
# speedup vs baseline: 1.1263x; 1.1263x over previous
"""CoverageAttention fused Trainium2 kernel (8 NeuronCores, data-parallel over batch).

Computation (per batch b):
  energy[s,h] = tanh( enc[b] @ W_h + dec_proj[b] + coverage[b,s]*W_c )
  scores[s]   = energy[s,:] @ v  (+ mask bias)
  attn        = softmax(scores); coverage_new = coverage + attn
  context     = attn @ enc[b]

v3 strategy. The per-call cost of executing this kernel through the PJRT path
is dominated by a fixed per-IO-TENSOR dispatch overhead (~11 ms/tensor,
measured), not by bytes or instruction count. So the kernel takes exactly ONE
input tensor (a packed bf16 blob holding the pre-transposed encoder, weights,
biases, coverage and mask rows) and produces ONE output tensor
(out[3, bpc, S] = ctx / attn / coverage_new rows).

Device-side (per core, 8 batches/core):
  - Encoder shipped once, in bf16, host-pre-tiled to per-(batch, s-block)
    contiguous [128, KT*SB] tiles so every DMA is a single 16 KiB/partition
    contiguous transfer.
  - Main matmul computes energy TRANSPOSED: psum[h:128, s:512] =
    sum_k wh[k].T @ encT[k] in bf16 (1 cycle/row on PE), plus one K=1 matmul
    adding coverage[s]*W_c[h] as an outer product.
  - tanh + dec_proj bias fused in one ScalarE activation per tile (dec_proj is
    computed on the host and shipped as a per-partition bias table).
  - scores accumulate on PE: psum[1,512] += v.T @ energy over 8 h-tiles, then
    one K=1 matmul adds the precomputed mask bias row ((mask-1)*1e4).
  - softmax on the [1, 2048] row: reduce_max(negate) -> Exp activation with
    bias=-max and fused accum_out denominator -> reciprocal -> scale.
  - context is computed WITHOUT re-reading the encoder: the batch's encT
    s-block tiles stay resident in SBUF (6-buffer pool); attn is broadcast to
    all 128 partitions with a K=1 PE matmul (ones ⊗ attn) and 64 DVE
    multiply + free-dim-reduce pairs produce ctxT[e], finished by a small PE
    transpose to lay the context out as a row.
  - Batches software-pipelined: tail(b) is emitted after main(b+1)'s first
    s-block so softmax+context hide under the next batch's matmuls while
    keeping the encoder-tile pool race-free.
"""

import numpy as np

P = 128
B_FULL = 64
S_FULL = 2048
H = 1024
E = 2048
N_CORES = 8
SB = 512           # s-block (matmul free dim / psum bank)
KT = E // P        # 16 contraction tiles over encoder dim
HT = H // P        # 8 h-tiles
NSB = S_FULL // SB # 4 s-blocks per batch
BPC = B_FULL // N_CORES

_CACHE: dict = {}


def _blob_offsets(bpc: int, S: int):
    """Element offsets of each logical block inside the packed bf16 blob."""
    nsb = S // SB
    off = {}
    cur = 0

    def add(name, n):
        nonlocal cur
        off[name] = cur
        cur += n

    add("enct", bpc * nsb * P * KT * SB)
    add("wh", P * KT * H)
    add("wc", H)
    add("vv", P * HT)
    add("dp", P * HT * bpc)
    add("covr", bpc * S)
    add("maskb", bpc * S)
    add("ones", P)
    add("ident", P * P)
    return off, cur


def _build_nc(bpc: int, S: int):
    import concourse.mybir as mybir
    import concourse.tile as tile
    from concourse import bacc

    f32 = mybir.dt.float32
    bf16 = mybir.dt.bfloat16
    Tanh = mybir.ActivationFunctionType.Tanh
    Exp = mybir.ActivationFunctionType.Exp
    Copy = mybir.ActivationFunctionType.Copy
    X = mybir.AxisListType.X

    nsb = S // SB
    off, total = _blob_offsets(bpc, S)

    # enable_partition_id=False: the kernel never reads the core id (all
    # per-core data is prepared host-side), and every NEFF io tensor costs
    # ~11 ms of per-call dispatch overhead on this setup.
    nc = bacc.Bacc(None, target_bir_lowering=False, enable_partition_id=False)

    blob = nc.dram_tensor("blob", [total], bf16, kind="ExternalInput")
    out_o = nc.dram_tensor("out_o", [3, bpc, S], f32, kind="ExternalOutput")

    def bslice(name, n, shape_expr, **axes):
        ap = blob[off[name]:off[name] + n]
        return ap.rearrange(shape_expr, **axes)

    with tile.TileContext(nc) as tc:
        with (
            tc.tile_pool(name="weights", bufs=1) as weights,
            tc.tile_pool(name="etp", bufs=6) as etp,
            tc.tile_pool(name="epool", bufs=2) as epool,
            tc.tile_pool(name="rows", bufs=1) as rows,
            tc.tile_pool(name="rows2", bufs=2) as rows2,
            tc.tile_pool(name="singles", bufs=1) as singles,
            tc.tile_pool(name="mainps", bufs=2, space="PSUM") as mainps,
            tc.tile_pool(name="scoresps", bufs=2, space="PSUM") as scoresps,
            tc.tile_pool(name="bcps", bufs=2, space="PSUM") as bcps,
            tc.tile_pool(name="ctxps", bufs=2, space="PSUM") as ctxps,
        ):
            # ---------------- prologue: weights ----------------
            # wh rides the scalar-engine HWDGE ring (TRN2 has two physical
            # rings) so it streams in parallel with the first encoder tiles on
            # the sync ring; chunked so early k-groups unblock sooner.
            wh_sb = weights.tile([P, KT, H], bf16, tag="wh")
            wh_src = bslice("wh", P * KT * H, "(p k h) -> p k h", p=P, h=H)
            for c in range(4):
                kc = KT // 4
                nc.scalar.dma_start(
                    wh_sb[:, c * kc:(c + 1) * kc, :],
                    wh_src[:, c * kc:(c + 1) * kc, :])
            wc_sb = singles.tile([1, H], bf16)
            nc.sync.dma_start(wc_sb, bslice("wc", H, "(o h) -> o h", o=1))
            v_sb = singles.tile([P, HT], bf16)
            nc.sync.dma_start(v_sb, bslice("vv", P * HT, "(p t) -> p t", p=P))
            one_sb = singles.tile([1, P], bf16)
            nc.sync.dma_start(one_sb, bslice("ones", P, "(o p) -> o p", o=1))
            dp_sb = singles.tile([P, HT, bpc], bf16)
            nc.sync.dma_start(
                dp_sb,
                bslice("dp", P * HT * bpc, "(p h b) -> p h b", p=P, b=bpc))
            id_sb = singles.tile([P, P], bf16)
            nc.sync.dma_start(
                id_sb, bslice("ident", P * P, "(p q) -> p q", p=P))
            # f32 identity (exact convert) so the ctx transpose is a pure-f32
            # matmul — the BIR verifier rejects mixed f32/bf16 matmul inputs.
            idf_sb = singles.tile([P, P], f32)
            nc.scalar.copy(idf_sb, id_sb)

            state = {}

            def emit_main_sblock(b, sb, ets):
                covr_row, maskb_row, scores_row = state[b][1:4]
                et = etp.tile([P, KT, SB], bf16, tag="et")
                nc.sync.dma_start(
                    et,
                    blob[off["enct"] + (b * nsb + sb) * P * KT * SB:
                         off["enct"] + (b * nsb + sb + 1) * P * KT * SB]
                    .rearrange("(p k s) -> p k s", p=P, s=SB),
                )
                ets.append(et)
                sc_ps = scoresps.tile([1, SB], f32, tag="scoresps")
                for ht in range(HT):
                    mp = mainps.tile([P, SB], f32, tag="mainps")
                    for k in range(KT):
                        nc.tensor.matmul(
                            mp,
                            wh_sb[:, k, ht * P:(ht + 1) * P],
                            et[:, k, :],
                            start=(k == 0),
                            stop=False,
                        )
                    # += coverage[s] * W_c[h]  (outer product, K=1)
                    nc.tensor.matmul(
                        mp,
                        wc_sb[0:1, ht * P:(ht + 1) * P],
                        covr_row[0:1, sb * SB:(sb + 1) * SB],
                        start=False,
                        stop=True,
                    )
                    en = epool.tile([P, SB], bf16, tag="energy")
                    nc.scalar.activation(
                        en, mp, Tanh, bias=dp_sb[:, ht, b:b + 1]
                    )
                    nc.tensor.matmul(
                        sc_ps,
                        v_sb[:, ht:ht + 1],
                        en,
                        start=(ht == 0),
                        stop=False,
                    )
                # += (mask-1)*1e4
                nc.tensor.matmul(
                    sc_ps,
                    one_sb[0:1, 0:1],
                    maskb_row[0:1, sb * SB:(sb + 1) * SB],
                    start=False,
                    stop=True,
                )
                nc.scalar.copy(scores_row[0:1, sb * SB:(sb + 1) * SB], sc_ps)

            def begin_main(b):
                # bufs=2: batch b+1's row DMAs must not wait on batch b's last
                # readers (a ~6 us PE stall per batch otherwise).
                covr_row = rows.tile([1, S], bf16, tag="covr", bufs=2)
                nc.sync.dma_start(
                    covr_row,
                    blob[off["covr"] + b * S: off["covr"] + (b + 1) * S]
                    .rearrange("(o s) -> o s", o=1))
                maskb_row = rows.tile([1, S], bf16, tag="maskb", bufs=2)
                nc.sync.dma_start(
                    maskb_row,
                    blob[off["maskb"] + b * S: off["maskb"] + (b + 1) * S]
                    .rearrange("(o s) -> o s", o=1))
                scores_row = rows2.tile([1, S], f32, tag="scores")
                ets = []
                state[b] = (covr_row, covr_row, maskb_row, scores_row, ets)

            def emit_tail(b):
                covr_row, _, _, scores_row, ets = state.pop(b)
                nmax = singles.tile([1, 1], f32, tag=f"nmax{b % 2}")
                nc.vector.reduce_max(nmax, scores_row[0:1, :], axis=X,
                                     negate=True)
                attn_u = rows.tile([1, S], f32, tag="attnu")
                den = singles.tile([1, 1], f32, tag=f"den{b % 2}")
                nc.scalar.activation(
                    attn_u, scores_row[0:1, :], Exp, bias=nmax[0:1, 0:1],
                    accum_out=den[0:1, 0:1],
                )
                rden = singles.tile([1, 1], f32, tag=f"rden{b % 2}")
                nc.vector.reciprocal(rden, den)
                attn_rb = rows.tile([1, S], bf16, tag="attnrb")
                nc.vector.tensor_scalar_mul(attn_rb, attn_u, rden[0:1, 0:1])
                # attn (f32): in-place scale of attn_u
                nc.vector.tensor_scalar_mul(attn_u, attn_u, rden[0:1, 0:1])
                nc.sync.dma_start(
                    out_o[1, b].rearrange("(o s) -> o s", o=1), attn_u)
                # coverage_new = coverage + attn
                covn_row = rows.tile([1, S], f32, tag="covn")
                nc.vector.tensor_add(covn_row, covr_row, attn_u)
                nc.sync.dma_start(
                    out_o[2, b].rearrange("(o s) -> o s", o=1), covn_row)
                # context from the SBUF-resident encT tiles: broadcast attn to
                # all 128 partitions via a K=1 PE matmul (ones ⊗ attn_row),
                # then per (kt, s-block): DVE multiply + free-dim reduce.
                pf = rows2.tile([P, KT, nsb], f32, tag="pf")
                for sb in range(nsb):
                    et = ets[sb]
                    bc = bcps.tile([P, SB], f32, tag="bcps")
                    nc.tensor.matmul(
                        bc,
                        one_sb[0:1, :],
                        attn_rb[0:1, sb * SB:(sb + 1) * SB],
                        start=True,
                        stop=True,
                    )
                    for kt in range(KT):
                        # multiply on DVE, free-dim reduce on ScalarE
                        # (activation Copy + accum_out) so the two engines
                        # pipeline and the last batch's drain is halved.
                        prod = rows.tile([P, SB], bf16, tag="ttscratch",
                                         bufs=3)
                        nc.vector.tensor_mul(prod, et[:, kt, :], bc)
                        dump = rows.tile([P, SB], bf16, tag="actdump")
                        nc.scalar.activation(
                            dump, prod, Copy,
                            accum_out=pf[:, kt, sb:sb + 1])
                ctxc = rows2.tile([P, KT], f32, tag="ctxc")
                nc.vector.reduce_sum(ctxc, pf[:, :, :], axis=X)
                ctp = ctxps.tile([KT, P], f32, tag="ctxps")
                nc.tensor.transpose(ctp, ctxc, idf_sb)
                ctx_sb = rows2.tile([KT, P], f32, tag="ctxsb")
                nc.scalar.copy(ctx_sb, ctp)
                nc.sync.dma_start(
                    out_o[0, b].rearrange("(k p) -> k p", k=KT), ctx_sb)

            # software pipeline: tail(b-1) lands after main(b)'s first s-block
            # so the et pool (6 bufs) never reuses a slot before its tail
            # readers are emitted.
            begin_main(0)
            for sb in range(nsb):
                emit_main_sblock(0, sb, state[0][4])
            for b in range(1, bpc):
                begin_main(b)
                emit_main_sblock(b, 0, state[b][4])
                emit_tail(b - 1)
                for sb in range(1, nsb):
                    emit_main_sblock(b, sb, state[b][4])
            emit_tail(bpc - 1)

    nc.compile()
    return nc


def _get_nc(bpc: int, S: int):
    key = (bpc, S)
    if key not in _CACHE:
        _CACHE[key] = _build_nc(bpc, S)
    return _CACHE[key]


def _prepare_in_maps(decoder_hidden, encoder_outputs, coverage, mask,
                     W_h, W_d, W_c, v, n_cores: int):
    """Host-side prep: shard over batch, pack one bf16 blob per core."""
    import ml_dtypes
    bf16 = ml_dtypes.bfloat16

    dec = np.asarray(decoder_hidden, dtype=np.float32)
    cov = np.asarray(coverage, dtype=np.float32)
    msk = np.asarray(mask)
    B = dec.shape[0]
    S = cov.shape[1]
    nsb = S // SB
    bpc = B // n_cores
    off, total = _blob_offsets(bpc, S)

    wh_np = np.asarray(W_h, dtype=np.float32)          # [E, H]
    wh_b = np.ascontiguousarray(
        wh_np.reshape(KT, P, H).transpose(1, 0, 2)).astype(bf16).ravel()
    wc_b = np.asarray(W_c, dtype=np.float32).astype(bf16).ravel()
    v_np = np.asarray(v, dtype=np.float32)[:, 0]
    v_b = np.ascontiguousarray(v_np.reshape(HT, P).T).astype(bf16).ravel()
    dec_proj = (dec.astype(np.float64) @ np.asarray(W_d, np.float64))
    dec_proj = dec_proj.astype(np.float32)             # [B, H]
    maskb = ((msk.astype(np.float32) - 1.0) * 10000.0).astype(bf16)
    ones_b = np.ones(P, dtype=np.float32).astype(bf16)
    ident_b = np.eye(P, dtype=np.float32).astype(bf16).ravel()

    enc = np.asarray(encoder_outputs, dtype=np.float32)
    in_maps = []
    for c in range(n_cores):
        sl = slice(c * bpc, (c + 1) * bpc)
        # [bpc, S, E] -> encT [bpc, E, S] -> [bpc, KT, P, nsb, SB]
        # -> [bpc, nsb, P, KT, SB]
        enct = (
            enc[sl]
            .transpose(0, 2, 1)
            .reshape(bpc, KT, P, nsb, SB)
            .transpose(0, 3, 2, 1, 4)
        )
        dslice = dec_proj[sl]                              # [bpc, H]
        dp_c = np.ascontiguousarray(
            dslice.T.reshape(HT, P, bpc).transpose(1, 0, 2))

        blob = np.empty(total, dtype=bf16)
        blob[off["enct"]:off["enct"] + enct.size] = (
            np.ascontiguousarray(enct).astype(bf16).ravel())
        blob[off["wh"]:off["wh"] + wh_b.size] = wh_b
        blob[off["wc"]:off["wc"] + wc_b.size] = wc_b
        blob[off["vv"]:off["vv"] + v_b.size] = v_b
        blob[off["dp"]:off["dp"] + dp_c.size] = (
            dp_c.astype(bf16).ravel())
        blob[off["covr"]:off["covr"] + bpc * S] = (
            cov[sl].astype(bf16).ravel())
        blob[off["maskb"]:off["maskb"] + bpc * S] = maskb[sl].ravel()
        blob[off["ones"]:off["ones"] + P] = ones_b
        blob[off["ident"]:off["ident"] + P * P] = ident_b
        in_maps.append({"blob": blob})
    return in_maps, bpc


def kernel(decoder_hidden, encoder_outputs, coverage, mask, W_h, W_d, W_c, v):
    from concourse.bass_utils import run_bass_kernel_spmd

    in_maps, bpc = _prepare_in_maps(
        decoder_hidden, encoder_outputs, coverage, mask, W_h, W_d, W_c, v,
        N_CORES,
    )
    S = np.asarray(coverage).shape[1]
    nc = _get_nc(bpc, S)
    res = run_bass_kernel_spmd(nc, in_maps, core_ids=list(range(N_CORES)))
    context = np.concatenate([r["out_o"][0] for r in res.results], axis=0)
    attn = np.concatenate([r["out_o"][1] for r in res.results], axis=0)
    covn = np.concatenate([r["out_o"][2] for r in res.results], axis=0)
    return context, attn, covn


# revision 22
# speedup vs baseline: 1.1478x; 1.0191x over previous
"""CoverageAttention fused Trainium2 kernel (8 NeuronCores, data-parallel over batch).

Computation (per batch b):
  energy[s,h] = tanh( enc[b] @ W_h + dec_proj[b] + coverage[b,s]*W_c )
  scores[s]   = energy[s,:] @ v  (+ mask bias)
  attn        = softmax(scores); coverage_new = coverage + attn
  context     = attn @ enc[b]

v3 strategy. The per-call cost of executing this kernel through the PJRT path
is dominated by a fixed per-IO-TENSOR dispatch overhead (~11 ms/tensor,
measured), not by bytes or instruction count. So the kernel takes exactly ONE
input tensor (a packed bf16 blob holding the pre-transposed encoder, weights,
biases, coverage and mask rows) and produces ONE output tensor
(out[3, bpc, S] = ctx / attn / coverage_new rows).

Device-side (per core, 8 batches/core):
  - Encoder shipped once, in bf16, host-pre-tiled to per-(batch, s-block)
    contiguous [128, KT*SB] tiles so every DMA is a single 16 KiB/partition
    contiguous transfer.
  - Main matmul computes energy TRANSPOSED: psum[h:128, s:512] =
    sum_k wh[k].T @ encT[k] in bf16 (1 cycle/row on PE), plus one K=1 matmul
    adding coverage[s]*W_c[h] as an outer product.
  - tanh + dec_proj bias fused in one ScalarE activation per tile (dec_proj is
    computed on the host and shipped as a per-partition bias table).
  - scores accumulate on PE: psum[1,512] += v.T @ energy over 8 h-tiles, then
    one K=1 matmul adds the precomputed mask bias row ((mask-1)*1e4).
  - softmax on the [1, 2048] row: reduce_max(negate) -> Exp activation with
    bias=-max and fused accum_out denominator -> reciprocal -> scale.
  - context is computed WITHOUT re-reading the encoder: the batch's encT
    s-block tiles stay resident in SBUF (6-buffer pool); attn is broadcast to
    all 128 partitions with a K=1 PE matmul (ones ⊗ attn) and 64 DVE
    multiply + free-dim-reduce pairs produce ctxT[e], finished by a small PE
    transpose to lay the context out as a row.
  - Batches software-pipelined: tail(b) is emitted after main(b+1)'s first
    s-block so softmax+context hide under the next batch's matmuls while
    keeping the encoder-tile pool race-free.
"""

import numpy as np

P = 128
B_FULL = 64
S_FULL = 2048
H = 1024
E = 2048
N_CORES = 8
SB = 512           # s-block (matmul free dim / psum bank)
KT = E // P        # 16 contraction tiles over encoder dim
HT = H // P        # 8 h-tiles
NSB = S_FULL // SB # 4 s-blocks per batch
BPC = B_FULL // N_CORES

_CACHE: dict = {}


def _blob_offsets(bpc: int, S: int):
    """Element offsets of each logical block inside the packed bf16 blob."""
    nsb = S // SB
    off = {}
    cur = 0

    def add(name, n):
        nonlocal cur
        off[name] = cur
        cur += n

    add("enct", bpc * nsb * P * KT * SB)
    add("wh", P * KT * H)
    add("wc", H)
    add("vv", P * HT)
    add("dp", P * HT * bpc)
    add("covr", bpc * S)
    add("maskb", bpc * S)
    add("ones", P)
    add("ident", P * P)
    return off, cur


def _build_nc(bpc: int, S: int):
    import concourse.mybir as mybir
    import concourse.tile as tile
    from concourse import bacc

    f32 = mybir.dt.float32
    bf16 = mybir.dt.bfloat16
    Tanh = mybir.ActivationFunctionType.Tanh
    Exp = mybir.ActivationFunctionType.Exp
    Copy = mybir.ActivationFunctionType.Copy
    X = mybir.AxisListType.X

    nsb = S // SB
    off, total = _blob_offsets(bpc, S)

    # enable_partition_id=False: the kernel never reads the core id (all
    # per-core data is prepared host-side), and every NEFF io tensor costs
    # ~11 ms of per-call dispatch overhead on this setup.
    nc = bacc.Bacc(None, target_bir_lowering=False, enable_partition_id=False)

    blob = nc.dram_tensor("blob", [total], bf16, kind="ExternalInput")
    out_o = nc.dram_tensor("out_o", [3, bpc, S], f32, kind="ExternalOutput")

    def bslice(name, n, shape_expr, **axes):
        ap = blob[off[name]:off[name] + n]
        return ap.rearrange(shape_expr, **axes)

    with tile.TileContext(nc) as tc:
        with (
            tc.tile_pool(name="weights", bufs=1) as weights,
            tc.tile_pool(name="etp", bufs=6) as etp,
            tc.tile_pool(name="epool", bufs=2) as epool,
            tc.tile_pool(name="rows", bufs=1) as rows,
            tc.tile_pool(name="rows2", bufs=2) as rows2,
            tc.tile_pool(name="singles", bufs=1) as singles,
            tc.tile_pool(name="mainps", bufs=2, space="PSUM") as mainps,
            tc.tile_pool(name="scoresps", bufs=2, space="PSUM") as scoresps,
            tc.tile_pool(name="bcps", bufs=2, space="PSUM") as bcps,
            tc.tile_pool(name="ctxps", bufs=2, space="PSUM") as ctxps,
        ):
            # ---------------- prologue: weights ----------------
            # wh rides the scalar-engine HWDGE ring (TRN2 has two physical
            # rings) so it streams in parallel with the first encoder tiles on
            # the sync ring; chunked so early k-groups unblock sooner.
            wh_sb = weights.tile([P, KT, H], bf16, tag="wh")
            wh_src = bslice("wh", P * KT * H, "(p k h) -> p k h", p=P, h=H)
            for c in range(4):
                kc = KT // 4
                nc.scalar.dma_start(
                    wh_sb[:, c * kc:(c + 1) * kc, :],
                    wh_src[:, c * kc:(c + 1) * kc, :])
            wc_sb = singles.tile([1, H], bf16)
            nc.sync.dma_start(wc_sb, bslice("wc", H, "(o h) -> o h", o=1))
            v_sb = singles.tile([P, HT], bf16)
            nc.sync.dma_start(v_sb, bslice("vv", P * HT, "(p t) -> p t", p=P))
            one_sb = singles.tile([1, P], bf16)
            nc.sync.dma_start(one_sb, bslice("ones", P, "(o p) -> o p", o=1))
            dp_sb = singles.tile([P, HT, bpc], bf16)
            nc.sync.dma_start(
                dp_sb,
                bslice("dp", P * HT * bpc, "(p h b) -> p h b", p=P, b=bpc))
            id_sb = singles.tile([P, P], bf16)
            nc.sync.dma_start(
                id_sb, bslice("ident", P * P, "(p q) -> p q", p=P))
            # f32 identity (exact convert) so the ctx transpose is a pure-f32
            # matmul — the BIR verifier rejects mixed f32/bf16 matmul inputs.
            idf_sb = singles.tile([P, P], f32)
            nc.scalar.copy(idf_sb, id_sb)

            state = {}

            def emit_main_sblock(b, sb, ets):
                covr_row, maskb_row, scores_row = state[b][1:4]
                et = etp.tile([P, KT, SB], bf16, tag="et")
                nc.sync.dma_start(
                    et,
                    blob[off["enct"] + (b * nsb + sb) * P * KT * SB:
                         off["enct"] + (b * nsb + sb + 1) * P * KT * SB]
                    .rearrange("(p k s) -> p k s", p=P, s=SB),
                )
                ets.append(et)
                sc_ps = scoresps.tile([1, SB], f32, tag="scoresps")
                for ht in range(HT):
                    mp = mainps.tile([P, SB], f32, tag="mainps")
                    for k in range(KT):
                        nc.tensor.matmul(
                            mp,
                            wh_sb[:, k, ht * P:(ht + 1) * P],
                            et[:, k, :],
                            start=(k == 0),
                            stop=False,
                        )
                    # += coverage[s] * W_c[h]  (outer product, K=1)
                    nc.tensor.matmul(
                        mp,
                        wc_sb[0:1, ht * P:(ht + 1) * P],
                        covr_row[0:1, sb * SB:(sb + 1) * SB],
                        start=False,
                        stop=True,
                    )
                    en = epool.tile([P, SB], bf16, tag="energy")
                    nc.scalar.activation(
                        en, mp, Tanh, bias=dp_sb[:, ht, b:b + 1]
                    )
                    nc.tensor.matmul(
                        sc_ps,
                        v_sb[:, ht:ht + 1],
                        en,
                        start=(ht == 0),
                        stop=False,
                    )
                # += (mask-1)*1e4
                nc.tensor.matmul(
                    sc_ps,
                    one_sb[0:1, 0:1],
                    maskb_row[0:1, sb * SB:(sb + 1) * SB],
                    start=False,
                    stop=True,
                )
                nc.scalar.copy(scores_row[0:1, sb * SB:(sb + 1) * SB], sc_ps)

            def begin_main(b):
                # bufs=2: batch b+1's row DMAs must not wait on batch b's last
                # readers (a ~6 us PE stall per batch otherwise).
                covr_row = rows.tile([1, S], bf16, tag="covr", bufs=2)
                nc.sync.dma_start(
                    covr_row,
                    blob[off["covr"] + b * S: off["covr"] + (b + 1) * S]
                    .rearrange("(o s) -> o s", o=1))
                maskb_row = rows.tile([1, S], bf16, tag="maskb", bufs=2)
                nc.sync.dma_start(
                    maskb_row,
                    blob[off["maskb"] + b * S: off["maskb"] + (b + 1) * S]
                    .rearrange("(o s) -> o s", o=1))
                scores_row = rows2.tile([1, S], f32, tag="scores")
                ets = []
                state[b] = (covr_row, covr_row, maskb_row, scores_row, ets)

            def emit_tail(b):
                covr_row, _, _, scores_row, ets = state.pop(b)
                nmax = singles.tile([1, 1], f32, tag=f"nmax{b % 2}")
                nc.vector.reduce_max(nmax, scores_row[0:1, :], axis=X,
                                     negate=True)
                attn_u = rows.tile([1, S], f32, tag="attnu")
                den = singles.tile([1, 1], f32, tag=f"den{b % 2}")
                nc.scalar.activation(
                    attn_u, scores_row[0:1, :], Exp, bias=nmax[0:1, 0:1],
                    accum_out=den[0:1, 0:1],
                )
                rden = singles.tile([1, 1], f32, tag=f"rden{b % 2}")
                nc.vector.reciprocal(rden, den)
                attn_rb = rows.tile([1, S], bf16, tag="attnrb")
                nc.vector.tensor_scalar_mul(attn_rb, attn_u, rden[0:1, 0:1])
                # attn (f32): in-place scale of attn_u
                nc.vector.tensor_scalar_mul(attn_u, attn_u, rden[0:1, 0:1])
                nc.sync.dma_start(
                    out_o[1, b].rearrange("(o s) -> o s", o=1), attn_u)
                # coverage_new = coverage + attn
                covn_row = rows.tile([1, S], f32, tag="covn")
                nc.vector.tensor_add(covn_row, covr_row, attn_u)
                nc.sync.dma_start(
                    out_o[2, b].rearrange("(o s) -> o s", o=1), covn_row)
                # context from the SBUF-resident encT tiles: broadcast attn to
                # all 128 partitions via a K=1 PE matmul (ones ⊗ attn_row),
                # then per (kt, s-block): DVE multiply + free-dim reduce.
                pf = rows2.tile([P, KT, nsb], f32, tag="pf")
                for sb in range(nsb):
                    et = ets[sb]
                    bc = bcps.tile([P, SB], f32, tag="bcps")
                    nc.tensor.matmul(
                        bc,
                        one_sb[0:1, :],
                        attn_rb[0:1, sb * SB:(sb + 1) * SB],
                        start=True,
                        stop=True,
                    )
                    for kt in range(KT):
                        # multiply on DVE, free-dim reduce on ScalarE
                        # (activation Copy + accum_out) so the two engines
                        # pipeline and the last batch's drain is halved.
                        prod = rows.tile([P, SB], bf16, tag="ttscratch",
                                         bufs=3)
                        nc.vector.tensor_mul(prod, et[:, kt, :], bc)
                        dump = rows.tile([P, SB], bf16, tag="actdump")
                        nc.scalar.activation(
                            dump, prod, Copy,
                            accum_out=pf[:, kt, sb:sb + 1])
                ctxc = rows2.tile([P, KT], f32, tag="ctxc")
                nc.vector.reduce_sum(ctxc, pf[:, :, :], axis=X)
                ctp = ctxps.tile([KT, P], f32, tag="ctxps")
                nc.tensor.transpose(ctp, ctxc, idf_sb)
                ctx_sb = rows2.tile([KT, P], f32, tag="ctxsb")
                nc.scalar.copy(ctx_sb, ctp)
                nc.sync.dma_start(
                    out_o[0, b].rearrange("(k p) -> k p", k=KT), ctx_sb)

            # software pipeline: tail(b-1) lands after main(b)'s first s-block
            # so the et pool (6 bufs) never reuses a slot before its tail
            # readers are emitted.
            begin_main(0)
            for sb in range(nsb):
                emit_main_sblock(0, sb, state[0][4])
            for b in range(1, bpc):
                begin_main(b)
                emit_main_sblock(b, 0, state[b][4])
                emit_tail(b - 1)
                for sb in range(1, nsb):
                    emit_main_sblock(b, sb, state[b][4])
            emit_tail(bpc - 1)

    nc.compile()
    return nc


def _get_nc(bpc: int, S: int):
    key = (bpc, S)
    if key not in _CACHE:
        _CACHE[key] = _build_nc(bpc, S)
    return _CACHE[key]


def _prepare_in_maps(decoder_hidden, encoder_outputs, coverage, mask,
                     W_h, W_d, W_c, v, n_cores: int):
    """Host-side prep: shard over batch, pack one bf16 blob per core."""
    import ml_dtypes
    bf16 = ml_dtypes.bfloat16

    dec = np.asarray(decoder_hidden, dtype=np.float32)
    cov = np.asarray(coverage, dtype=np.float32)
    msk = np.asarray(mask)
    B = dec.shape[0]
    S = cov.shape[1]
    nsb = S // SB
    bpc = B // n_cores
    off, total = _blob_offsets(bpc, S)

    wh_np = np.asarray(W_h, dtype=np.float32)          # [E, H]
    wh_b = np.ascontiguousarray(
        wh_np.reshape(KT, P, H).transpose(1, 0, 2)).astype(bf16).ravel()
    wc_b = np.asarray(W_c, dtype=np.float32).astype(bf16).ravel()
    v_np = np.asarray(v, dtype=np.float32)[:, 0]
    v_b = np.ascontiguousarray(v_np.reshape(HT, P).T).astype(bf16).ravel()
    dec_proj = (dec.astype(np.float64) @ np.asarray(W_d, np.float64))
    dec_proj = dec_proj.astype(np.float32)             # [B, H]
    maskb = ((msk.astype(np.float32) - 1.0) * 10000.0).astype(bf16)
    ones_b = np.ones(P, dtype=np.float32).astype(bf16)
    ident_b = np.eye(P, dtype=np.float32).astype(bf16).ravel()

    enc = np.asarray(encoder_outputs, dtype=np.float32)
    in_maps = []
    for c in range(n_cores):
        sl = slice(c * bpc, (c + 1) * bpc)
        # [bpc, S, E] -> encT [bpc, E, S] -> [bpc, KT, P, nsb, SB]
        # -> [bpc, nsb, P, KT, SB]
        enct = (
            enc[sl]
            .transpose(0, 2, 1)
            .reshape(bpc, KT, P, nsb, SB)
            .transpose(0, 3, 2, 1, 4)
        )
        dslice = dec_proj[sl]                              # [bpc, H]
        dp_c = np.ascontiguousarray(
            dslice.T.reshape(HT, P, bpc).transpose(1, 0, 2))

        blob = np.empty(total, dtype=bf16)
        blob[off["enct"]:off["enct"] + enct.size] = (
            np.ascontiguousarray(enct).astype(bf16).ravel())
        blob[off["wh"]:off["wh"] + wh_b.size] = wh_b
        blob[off["wc"]:off["wc"] + wc_b.size] = wc_b
        blob[off["vv"]:off["vv"] + v_b.size] = v_b
        blob[off["dp"]:off["dp"] + dp_c.size] = (
            dp_c.astype(bf16).ravel())
        blob[off["covr"]:off["covr"] + bpc * S] = (
            cov[sl].astype(bf16).ravel())
        blob[off["maskb"]:off["maskb"] + bpc * S] = maskb[sl].ravel()
        blob[off["ones"]:off["ones"] + P] = ones_b
        blob[off["ident"]:off["ident"] + P * P] = ident_b
        in_maps.append({"blob": blob})
    return in_maps, bpc


def kernel(decoder_hidden, encoder_outputs, coverage, mask, W_h, W_d, W_c, v):
    from concourse.bass_utils import run_bass_kernel_spmd

    in_maps, bpc = _prepare_in_maps(
        decoder_hidden, encoder_outputs, coverage, mask, W_h, W_d, W_c, v,
        N_CORES,
    )
    S = np.asarray(coverage).shape[1]
    nc = _get_nc(bpc, S)
    res = run_bass_kernel_spmd(nc, in_maps, core_ids=list(range(N_CORES)))
    context = np.concatenate([r["out_o"][0] for r in res.results], axis=0)
    attn = np.concatenate([r["out_o"][1] for r in res.results], axis=0)
    covn = np.concatenate([r["out_o"][2] for r in res.results], axis=0)
    return context, attn, covn


# revision 26
# speedup vs baseline: 1.1897x; 1.0364x over previous
"""CoverageAttention fused Trainium2 kernel (8 NeuronCores, data-parallel over batch).

Computation (per batch b):
  energy[s,h] = tanh( enc[b] @ W_h + dec_proj[b] + coverage[b,s]*W_c )
  scores[s]   = energy[s,:] @ v  (+ mask bias)
  attn        = softmax(scores); coverage_new = coverage + attn
  context     = attn @ enc[b]

v3 strategy. The per-call cost of executing this kernel through the PJRT path
is dominated by a fixed per-IO-TENSOR dispatch overhead (~11 ms/tensor,
measured), not by bytes or instruction count. So the kernel takes exactly ONE
input tensor (a packed bf16 blob holding the pre-transposed encoder, weights,
biases, coverage and mask rows) and produces ONE output tensor
(out[3, bpc, S] = ctx / attn / coverage_new rows).

Device-side (per core, 8 batches/core):
  - Encoder shipped once, in bf16, host-pre-tiled to per-(batch, s-block)
    contiguous [128, KT*SB] tiles so every DMA is a single 16 KiB/partition
    contiguous transfer.
  - Main matmul computes energy TRANSPOSED: psum[h:128, s:512] =
    sum_k wh[k].T @ encT[k] in bf16 (1 cycle/row on PE), plus one K=1 matmul
    adding coverage[s]*W_c[h] as an outer product.
  - tanh + dec_proj bias fused in one ScalarE activation per tile (dec_proj is
    computed on the host and shipped as a per-partition bias table).
  - scores accumulate on PE: psum[1,512] += v.T @ energy over 8 h-tiles, then
    one K=1 matmul adds the precomputed mask bias row ((mask-1)*1e4).
  - softmax on the [1, 2048] row: reduce_max(negate) -> Exp activation with
    bias=-max and fused accum_out denominator -> reciprocal -> scale.
  - context is computed WITHOUT re-reading the encoder: the batch's encT
    s-block tiles stay resident in SBUF (6-buffer pool); attn is broadcast to
    all 128 partitions with a K=1 PE matmul (ones ⊗ attn) and 64 DVE
    multiply + free-dim-reduce pairs produce ctxT[e], finished by a small PE
    transpose to lay the context out as a row.
  - Batches software-pipelined: tail(b) is emitted after main(b+1)'s first
    s-block so softmax+context hide under the next batch's matmuls while
    keeping the encoder-tile pool race-free.
"""

import numpy as np

P = 128
B_FULL = 64
S_FULL = 2048
H = 1024
E = 2048
N_CORES = 8
SB = 512           # s-block (matmul free dim / psum bank)
KT = E // P        # 16 contraction tiles over encoder dim
HT = H // P        # 8 h-tiles
NSB = S_FULL // SB # 4 s-blocks per batch
BPC = B_FULL // N_CORES

_CACHE: dict = {}


def _blob_offsets(bpc: int, S: int):
    """Element offsets of each logical block inside the packed bf16 blob."""
    nsb = S // SB
    off = {}
    cur = 0

    def add(name, n):
        nonlocal cur
        off[name] = cur
        cur += n

    add("enct", bpc * nsb * P * KT * SB)
    add("wh", P * KT * H)
    add("wc", H)
    add("vv", P * HT)
    add("dp", P * HT * bpc)
    add("covr", bpc * S)
    add("maskb", bpc * S)
    add("ones", P)
    add("ident", P * P)
    return off, cur


def _build_nc(bpc: int, S: int):
    import concourse.mybir as mybir
    import concourse.tile as tile
    from concourse import bacc

    f32 = mybir.dt.float32
    bf16 = mybir.dt.bfloat16
    Tanh = mybir.ActivationFunctionType.Tanh
    Exp = mybir.ActivationFunctionType.Exp
    Copy = mybir.ActivationFunctionType.Copy
    X = mybir.AxisListType.X

    nsb = S // SB
    off, total = _blob_offsets(bpc, S)

    # enable_partition_id=False: the kernel never reads the core id (all
    # per-core data is prepared host-side), and every NEFF io tensor costs
    # ~11 ms of per-call dispatch overhead on this setup.
    nc = bacc.Bacc(None, target_bir_lowering=False, enable_partition_id=False)

    blob = nc.dram_tensor("blob", [total], bf16, kind="ExternalInput")
    out_o = nc.dram_tensor("out_o", [3, bpc, S], f32, kind="ExternalOutput")

    def bslice(name, n, shape_expr, **axes):
        ap = blob[off[name]:off[name] + n]
        return ap.rearrange(shape_expr, **axes)

    with tile.TileContext(nc) as tc:
        with (
            tc.tile_pool(name="weights", bufs=1) as weights,
            tc.tile_pool(name="etp", bufs=6) as etp,
            tc.tile_pool(name="epool", bufs=2) as epool,
            tc.tile_pool(name="rows", bufs=1) as rows,
            tc.tile_pool(name="rows2", bufs=2) as rows2,
            tc.tile_pool(name="singles", bufs=1) as singles,
            tc.tile_pool(name="mainps", bufs=2, space="PSUM") as mainps,
            tc.tile_pool(name="scoresps", bufs=2, space="PSUM") as scoresps,
            tc.tile_pool(name="bcps", bufs=2, space="PSUM") as bcps,
            tc.tile_pool(name="ctxps", bufs=2, space="PSUM") as ctxps,
        ):
            def fetch_et(b, sb):
                et = etp.tile([P, KT, SB], bf16, tag="et")
                nc.sync.dma_start(
                    et,
                    blob[off["enct"] + (b * nsb + sb) * P * KT * SB:
                         off["enct"] + (b * nsb + sb + 1) * P * KT * SB]
                    .rearrange("(p k s) -> p k s", p=P, s=SB),
                )
                return et

            # ---------------- prologue: weights ----------------
            # wh rides the scalar-engine HWDGE ring (TRN2 has two physical
            # rings) so it streams in parallel with the first encoder tiles on
            # the sync ring; chunked so early k-groups unblock sooner.
            wh_sb = weights.tile([P, KT, H], bf16, tag="wh")
            wh_src = bslice("wh", P * KT * H, "(p k h) -> p k h", p=P, h=H)
            for c in range(4):
                kc = KT // 4
                nc.scalar.dma_start(
                    wh_sb[:, c * kc:(c + 1) * kc, :],
                    wh_src[:, c * kc:(c + 1) * kc, :])
            # first encoder tile leads the sync-ring queue so the 2 MB
            # transfer overlaps the weight chunks instead of queueing behind
            # the small prologue DMAs.
            et00 = fetch_et(0, 0)
            wc_sb = singles.tile([1, H], bf16)
            nc.sync.dma_start(wc_sb, bslice("wc", H, "(o h) -> o h", o=1))
            v_sb = singles.tile([P, HT], bf16)
            nc.sync.dma_start(v_sb, bslice("vv", P * HT, "(p t) -> p t", p=P))
            one_sb = singles.tile([1, P], bf16)
            nc.sync.dma_start(one_sb, bslice("ones", P, "(o p) -> o p", o=1))
            dp_sb = singles.tile([P, HT, bpc], bf16)
            nc.sync.dma_start(
                dp_sb,
                bslice("dp", P * HT * bpc, "(p h b) -> p h b", p=P, b=bpc))
            id_sb = singles.tile([P, P], bf16)
            nc.sync.dma_start(
                id_sb, bslice("ident", P * P, "(p q) -> p q", p=P))
            # f32 identity (exact convert) so the ctx transpose is a pure-f32
            # matmul — the BIR verifier rejects mixed f32/bf16 matmul inputs.
            idf_sb = singles.tile([P, P], f32)
            nc.scalar.copy(idf_sb, id_sb)

            state = {}

            def emit_main_sblock(b, sb, ets, et=None):
                covr_row, maskb_row, scores_row = state[b][1:4]
                if et is None:
                    et = fetch_et(b, sb)
                ets.append(et)
                sc_ps = scoresps.tile([1, SB], f32, tag="scoresps")
                for ht in range(HT):
                    mp = mainps.tile([P, SB], f32, tag="mainps")
                    for k in range(KT):
                        nc.tensor.matmul(
                            mp,
                            wh_sb[:, k, ht * P:(ht + 1) * P],
                            et[:, k, :],
                            start=(k == 0),
                            stop=False,
                        )
                    # += coverage[s] * W_c[h]  (outer product, K=1)
                    nc.tensor.matmul(
                        mp,
                        wc_sb[0:1, ht * P:(ht + 1) * P],
                        covr_row[0:1, sb * SB:(sb + 1) * SB],
                        start=False,
                        stop=True,
                    )
                    en = epool.tile([P, SB], bf16, tag="energy")
                    nc.scalar.activation(
                        en, mp, Tanh, bias=dp_sb[:, ht, b:b + 1]
                    )
                    nc.tensor.matmul(
                        sc_ps,
                        v_sb[:, ht:ht + 1],
                        en,
                        start=(ht == 0),
                        stop=False,
                    )
                # += (mask-1)*1e4
                nc.tensor.matmul(
                    sc_ps,
                    one_sb[0:1, 0:1],
                    maskb_row[0:1, sb * SB:(sb + 1) * SB],
                    start=False,
                    stop=True,
                )
                nc.scalar.copy(scores_row[0:1, sb * SB:(sb + 1) * SB], sc_ps)

            def begin_main(b):
                # bufs=2: batch b+1's row DMAs must not wait on batch b's last
                # readers (a ~6 us PE stall per batch otherwise).
                covr_row = rows.tile([1, S], bf16, tag="covr", bufs=2)
                nc.sync.dma_start(
                    covr_row,
                    blob[off["covr"] + b * S: off["covr"] + (b + 1) * S]
                    .rearrange("(o s) -> o s", o=1))
                maskb_row = rows.tile([1, S], bf16, tag="maskb", bufs=2)
                nc.sync.dma_start(
                    maskb_row,
                    blob[off["maskb"] + b * S: off["maskb"] + (b + 1) * S]
                    .rearrange("(o s) -> o s", o=1))
                scores_row = rows2.tile([1, S], f32, tag="scores")
                ets = []
                state[b] = (covr_row, covr_row, maskb_row, scores_row, ets)

            def emit_tail(b):
                covr_row, _, _, scores_row, ets = state.pop(b)
                nmax = singles.tile([1, 1], f32, tag=f"nmax{b % 2}")
                nc.vector.reduce_max(nmax, scores_row[0:1, :], axis=X,
                                     negate=True)
                attn_u = rows.tile([1, S], f32, tag="attnu")
                den = singles.tile([1, 1], f32, tag=f"den{b % 2}")
                nc.scalar.activation(
                    attn_u, scores_row[0:1, :], Exp, bias=nmax[0:1, 0:1],
                    accum_out=den[0:1, 0:1],
                )
                rden = singles.tile([1, 1], f32, tag=f"rden{b % 2}")
                nc.vector.reciprocal(rden, den)
                attn_rb = rows.tile([1, S], bf16, tag="attnrb")
                nc.vector.tensor_scalar_mul(attn_rb, attn_u, rden[0:1, 0:1])
                # attn (f32): in-place scale of attn_u
                nc.vector.tensor_scalar_mul(attn_u, attn_u, rden[0:1, 0:1])
                nc.sync.dma_start(
                    out_o[1, b].rearrange("(o s) -> o s", o=1), attn_u)
                # coverage_new = coverage + attn
                covn_row = rows.tile([1, S], f32, tag="covn")
                nc.vector.tensor_add(covn_row, covr_row, attn_u)
                nc.sync.dma_start(
                    out_o[2, b].rearrange("(o s) -> o s", o=1), covn_row)
                # context from the SBUF-resident encT tiles: broadcast attn to
                # all 128 partitions via a K=1 PE matmul (ones ⊗ attn_row),
                # then per (kt, s-block): DVE multiply + free-dim reduce.
                pf = rows2.tile([P, KT, nsb], f32, tag="pf")
                for sb in range(nsb):
                    et = ets[sb]
                    bc = bcps.tile([P, SB], f32, tag="bcps")
                    nc.tensor.matmul(
                        bc,
                        one_sb[0:1, :],
                        attn_rb[0:1, sb * SB:(sb + 1) * SB],
                        start=True,
                        stop=True,
                    )
                    for kt in range(KT):
                        # multiply on DVE, free-dim reduce on ScalarE
                        # (activation Copy + accum_out) so the two engines
                        # pipeline and the last batch's drain is halved.
                        prod = rows.tile([P, SB], bf16, tag="ttscratch",
                                         bufs=3)
                        nc.vector.tensor_mul(prod, et[:, kt, :], bc)
                        dump = rows.tile([P, SB], bf16, tag="actdump")
                        nc.scalar.activation(
                            dump, prod, Copy,
                            accum_out=pf[:, kt, sb:sb + 1])
                ctxc = rows2.tile([P, KT], f32, tag="ctxc")
                nc.vector.reduce_sum(ctxc, pf[:, :, :], axis=X)
                ctp = ctxps.tile([KT, P], f32, tag="ctxps")
                nc.tensor.transpose(ctp, ctxc, idf_sb)
                ctx_sb = rows2.tile([KT, P], f32, tag="ctxsb")
                nc.scalar.copy(ctx_sb, ctp)
                nc.sync.dma_start(
                    out_o[0, b].rearrange("(k p) -> k p", k=KT), ctx_sb)

            # software pipeline: tail(b-1) lands after main(b)'s first s-block
            # so the et pool (6 bufs) never reuses a slot before its tail
            # readers are emitted.
            begin_main(0)
            for sb in range(nsb):
                emit_main_sblock(0, sb, state[0][4],
                                 et=et00 if sb == 0 else None)
            for b in range(1, bpc):
                begin_main(b)
                emit_main_sblock(b, 0, state[b][4])
                emit_tail(b - 1)
                for sb in range(1, nsb):
                    emit_main_sblock(b, sb, state[b][4])
            emit_tail(bpc - 1)

    nc.compile()
    return nc


def _get_nc(bpc: int, S: int):
    key = (bpc, S)
    if key not in _CACHE:
        _CACHE[key] = _build_nc(bpc, S)
    return _CACHE[key]


def _prepare_in_maps(decoder_hidden, encoder_outputs, coverage, mask,
                     W_h, W_d, W_c, v, n_cores: int):
    """Host-side prep: shard over batch, pack one bf16 blob per core."""
    import ml_dtypes
    bf16 = ml_dtypes.bfloat16

    dec = np.asarray(decoder_hidden, dtype=np.float32)
    cov = np.asarray(coverage, dtype=np.float32)
    msk = np.asarray(mask)
    B = dec.shape[0]
    S = cov.shape[1]
    nsb = S // SB
    bpc = B // n_cores
    off, total = _blob_offsets(bpc, S)

    wh_np = np.asarray(W_h, dtype=np.float32)          # [E, H]
    wh_b = np.ascontiguousarray(
        wh_np.reshape(KT, P, H).transpose(1, 0, 2)).astype(bf16).ravel()
    wc_b = np.asarray(W_c, dtype=np.float32).astype(bf16).ravel()
    v_np = np.asarray(v, dtype=np.float32)[:, 0]
    v_b = np.ascontiguousarray(v_np.reshape(HT, P).T).astype(bf16).ravel()
    dec_proj = (dec.astype(np.float64) @ np.asarray(W_d, np.float64))
    dec_proj = dec_proj.astype(np.float32)             # [B, H]
    maskb = ((msk.astype(np.float32) - 1.0) * 10000.0).astype(bf16)
    ones_b = np.ones(P, dtype=np.float32).astype(bf16)
    ident_b = np.eye(P, dtype=np.float32).astype(bf16).ravel()

    enc = np.asarray(encoder_outputs, dtype=np.float32)
    in_maps = []
    for c in range(n_cores):
        sl = slice(c * bpc, (c + 1) * bpc)
        # [bpc, S, E] -> encT [bpc, E, S] -> [bpc, KT, P, nsb, SB]
        # -> [bpc, nsb, P, KT, SB]
        enct = (
            enc[sl]
            .transpose(0, 2, 1)
            .reshape(bpc, KT, P, nsb, SB)
            .transpose(0, 3, 2, 1, 4)
        )
        dslice = dec_proj[sl]                              # [bpc, H]
        dp_c = np.ascontiguousarray(
            dslice.T.reshape(HT, P, bpc).transpose(1, 0, 2))

        blob = np.empty(total, dtype=bf16)
        blob[off["enct"]:off["enct"] + enct.size] = (
            np.ascontiguousarray(enct).astype(bf16).ravel())
        blob[off["wh"]:off["wh"] + wh_b.size] = wh_b
        blob[off["wc"]:off["wc"] + wc_b.size] = wc_b
        blob[off["vv"]:off["vv"] + v_b.size] = v_b
        blob[off["dp"]:off["dp"] + dp_c.size] = (
            dp_c.astype(bf16).ravel())
        blob[off["covr"]:off["covr"] + bpc * S] = (
            cov[sl].astype(bf16).ravel())
        blob[off["maskb"]:off["maskb"] + bpc * S] = maskb[sl].ravel()
        blob[off["ones"]:off["ones"] + P] = ones_b
        blob[off["ident"]:off["ident"] + P * P] = ident_b
        in_maps.append({"blob": blob})
    return in_maps, bpc


def kernel(decoder_hidden, encoder_outputs, coverage, mask, W_h, W_d, W_c, v):
    from concourse.bass_utils import run_bass_kernel_spmd

    in_maps, bpc = _prepare_in_maps(
        decoder_hidden, encoder_outputs, coverage, mask, W_h, W_d, W_c, v,
        N_CORES,
    )
    S = np.asarray(coverage).shape[1]
    nc = _get_nc(bpc, S)
    res = run_bass_kernel_spmd(nc, in_maps, core_ids=list(range(N_CORES)))
    context = np.concatenate([r["out_o"][0] for r in res.results], axis=0)
    attn = np.concatenate([r["out_o"][1] for r in res.results], axis=0)
    covn = np.concatenate([r["out_o"][2] for r in res.results], axis=0)
    return context, attn, covn
